# revision 10
# baseline (speedup 1.0000x reference)
"""Single-head MHA (QKV proj + softmax attention) on 8 Trainium2 cores.

Problem: x[8, 4096, 256] f32; per-batch attention with per-head emb 256.
Sharding: data-parallel — one batch element per NeuronCore (8 cores).

Per-core algorithm (S=4096, E=256, P=128 partitions):
  - transpose x -> xT[d, s] (PE transpose, bf16)
  - QT[e, s] = Wq @ xT (+bq), KT likewise, V[s, e] = xT.T @ WvT  (bf16)
  - per q-block of 512 columns, loop k-tiles of 128 rows:
      S^T[k, q] = KT_slice.T @ QT  (2 matmuls, fp32 PSUM)
      E[k, q]   = exp(S^T / 16)    (ScalarE, scale fused, bf16 out)
      outT[e,q]+= V_slice.T @ E    (2 matmuls, fp32 PSUM accumulation)
      denom    += E                (VectorE, fp32 accumulator)
    softmax denominators: PE-transpose denom blocks + free-dim reduce,
    reciprocal; out blocks: PE-transpose outT, scale by recip (per-partition
    broadcast), + bv (softmax rows sum to 1, so attn @ (V + bv) = attn@V + bv).

No running-max subtraction: scores/16 ~ N(0,1); max over 16.8M draws < ~6,
exp stays well inside fp32/bf16 range.
"""

from contextlib import ExitStack

import numpy as np

import concourse.bass as bass
import concourse.tile as tile
from concourse import bacc
from concourse import mybir
from concourse import bass_utils
from concourse.masks import make_identity

P = 128          # partitions
EMB = 256        # head dim
S = 4096         # sequence length
B = 8            # batch == number of cores
QB = 512         # q-block (free dim of S^T / E tiles)

F32 = mybir.dt.float32
BF16 = mybir.dt.bfloat16
AF = mybir.ActivationFunctionType


def _build(nc: bass.Bass, s_len: int = S) -> None:
    """Emit the per-core program into `nc` (SPMD: same program all cores)."""
    x = nc.dram_tensor("x", (s_len, EMB), F32, kind="ExternalInput").ap()
    Wq = nc.dram_tensor("Wq", (EMB, EMB), F32, kind="ExternalInput").ap()
    bq = nc.dram_tensor("bq", (EMB,), F32, kind="ExternalInput").ap()
    Wk = nc.dram_tensor("Wk", (EMB, EMB), F32, kind="ExternalInput").ap()
    bk = nc.dram_tensor("bk", (EMB,), F32, kind="ExternalInput").ap()
    Wv = nc.dram_tensor("Wv", (EMB, EMB), F32, kind="ExternalInput").ap()
    bv = nc.dram_tensor("bv", (EMB,), F32, kind="ExternalInput").ap()
    out = nc.dram_tensor("out", (s_len, EMB), F32, kind="ExternalOutput").ap()

    n_st = s_len // P      # 128-row tiles of the sequence
    n_qb = s_len // QB     # q-blocks
    n_kt = s_len // P      # k-tiles
    n_qt = QB // P         # 128-row q-tiles per q-block
    scale = float(EMB) ** -0.5

    with tile.TileContext(nc) as tc, ExitStack() as ctx:
        consts = ctx.enter_context(tc.tile_pool(name="consts", bufs=1))
        persist = ctx.enter_context(tc.tile_pool(name="persist", bufs=1))
        stage = ctx.enter_context(tc.tile_pool(name="stage", bufs=3))
        work = ctx.enter_context(tc.tile_pool(name="work", bufs=3))
        outp = ctx.enter_context(tc.tile_pool(name="outp", bufs=4))
        ps_mm = ctx.enter_context(tc.tile_pool(name="ps_mm", bufs=2, space="PSUM"))
        ps_acc = ctx.enter_context(tc.tile_pool(name="ps_acc", bufs=2, space="PSUM"))
        ps_tp = ctx.enter_context(tc.tile_pool(name="ps_tp", bufs=2, space="PSUM"))

        # identity for PE transposes (all transposes run in f32; the cast to
        # bf16 happens on the PSUM->SBUF copy)
        idf = consts.tile([P, P], F32)
        make_identity(nc, idf)

        # biases: bq/bk as per-partition columns (e on partitions),
        # bv broadcast across partitions (added at the very end).
        bq_sb = consts.tile([P, 2], F32)
        nc.gpsimd.dma_start(bq_sb, bq.rearrange("(t p) -> p t", p=P))
        bk_sb = consts.tile([P, 2], F32)
        nc.gpsimd.dma_start(bk_sb, bk.rearrange("(t p) -> p t", p=P))
        bv_bc = consts.tile([P, EMB], F32)
        nc.gpsimd.dma_start(
            bv_bc,
            bass.AP(tensor=bv.tensor, offset=bv.offset, ap=[[0, P], list(bv.ap[0])]),
        )

        # ---- weights: load W[e,d], cast bf16, PE-transpose -> WT[d,e] ----
        WT = {}
        for wname, wap in (("q", Wq), ("k", Wk), ("v", Wv)):
            wt0 = persist.tile([P, EMB], BF16, name=f"wt_{wname}_0")
            wt1 = persist.tile([P, EMB], BF16, name=f"wt_{wname}_1")
            WT[wname] = (wt0, wt1)
            for et in range(2):
                wst = stage.tile([P, EMB], F32, tag="wst")
                nc.sync.dma_start(wst, wap[et * P:(et + 1) * P, :])
                for dc in range(2):
                    tp = ps_tp.tile([P, P], F32, tag="tp")
                    nc.tensor.transpose(tp, wst[:, dc * P:(dc + 1) * P], idf)
                    nc.vector.tensor_copy(
                        WT[wname][dc][:, et * P:(et + 1) * P], tp
                    )

        # ---- x: load, cast bf16, PE-transpose -> xT[dc][d, s] ----
        xT = [persist.tile([P, s_len], BF16, name=f"xT{dc}") for dc in range(2)]
        for st_i in range(n_st):
            xst = stage.tile([P, EMB], F32, tag="xst")
            nc.sync.dma_start(xst, x[st_i * P:(st_i + 1) * P, :])
            for dc in range(2):
                tp = ps_tp.tile([P, P], F32, tag="tp")
                nc.tensor.transpose(tp, xst[:, dc * P:(dc + 1) * P], idf)
                nc.vector.tensor_copy(xT[dc][:, st_i * P:(st_i + 1) * P], tp)

        # ---- projections: QT/KT[e, s] (e on partitions), V[s, e] ----
        QT = [persist.tile([P, s_len], BF16, name=f"QT{t}") for t in range(2)]
        KT = [persist.tile([P, s_len], BF16, name=f"KT{t}") for t in range(2)]
        Vb = persist.tile([P, n_st, EMB], BF16, name="Vb")
        for t in range(2):
            for sb in range(n_qb):
                ssl = slice(sb * QB, (sb + 1) * QB)
                qps = ps_mm.tile([P, QB], F32, tag="mm")
                nc.tensor.matmul(qps, WT["q"][0][:, t * P:(t + 1) * P],
                                 xT[0][:, ssl], start=True, stop=False)
                nc.tensor.matmul(qps, WT["q"][1][:, t * P:(t + 1) * P],
                                 xT[1][:, ssl], start=False, stop=True)
                nc.scalar.activation(QT[t][:, ssl], qps, AF.Identity,
                                     bias=bq_sb[:, t:t + 1], scale=1.0)
                kps = ps_mm.tile([P, QB], F32, tag="mm")
                nc.tensor.matmul(kps, WT["k"][0][:, t * P:(t + 1) * P],
                                 xT[0][:, ssl], start=True, stop=False)
                nc.tensor.matmul(kps, WT["k"][1][:, t * P:(t + 1) * P],
                                 xT[1][:, ssl], start=False, stop=True)
                nc.scalar.activation(KT[t][:, ssl], kps, AF.Identity,
                                     bias=bk_sb[:, t:t + 1], scale=1.0)
        for st_i in range(n_st):
            vps = ps_mm.tile([P, EMB], F32, tag="mm")
            nc.tensor.matmul(vps, xT[0][:, st_i * P:(st_i + 1) * P], WT["v"][0],
                             start=True, stop=False)
            nc.tensor.matmul(vps, xT[1][:, st_i * P:(st_i + 1) * P], WT["v"][1],
                             start=False, stop=True)
            nc.scalar.activation(Vb[:, st_i, :], vps, AF.Copy)

        # ---- attention ----
        for qb_i in range(n_qb):
            qsl = slice(qb_i * QB, (qb_i + 1) * QB)
            po = [ps_acc.tile([P, QB], F32, tag="po", name=f"po{c}_{qb_i}")
                  for c in range(2)]
            dacc = work.tile([P, QB], F32, tag="dacc")
            e_prev = None
            for kt_i in range(n_kt):
                ksl = slice(kt_i * P, (kt_i + 1) * P)
                stp = ps_mm.tile([P, QB], F32, tag="mm")
                nc.tensor.matmul(stp, KT[0][:, ksl], QT[0][:, qsl],
                                 start=True, stop=False)
                nc.tensor.matmul(stp, KT[1][:, ksl], QT[1][:, qsl],
                                 start=False, stop=True)
                if e_prev is not None:
                    # PV for the previous k-tile: keeps PE busy while ACT
                    # computes this tile's exp (software pipelining).
                    kp = kt_i - 1
                    for c in range(2):
                        nc.tensor.matmul(po[c], Vb[:, kp, c * P:(c + 1) * P],
                                         e_prev, start=(kp == 0), stop=False)
                ebf = work.tile([P, QB], BF16, tag="E")
                nc.scalar.activation(ebf, stp, AF.Exp, scale=scale)
                if kt_i == 0:
                    nc.vector.tensor_copy(dacc, ebf)
                else:
                    nc.vector.tensor_add(dacc, dacc, ebf)
                e_prev = ebf
            for c in range(2):
                nc.tensor.matmul(po[c], Vb[:, n_kt - 1, c * P:(c + 1) * P],
                                 e_prev, start=(n_kt == 1), stop=True)

            # softmax denominators: transpose dacc 128-blocks, reduce along
            # free dim (the k remainder), then reciprocal -> [q, 1] columns.
            dsum = work.tile([P, n_qt], F32, tag="dsum")
            for j in range(n_qt):
                dtp = ps_tp.tile([P, P], F32, tag="tp")
                nc.tensor.transpose(dtp, dacc[:, j * P:(j + 1) * P], idf)
                nc.vector.reduce_sum(dsum[:, j:j + 1], dtp,
                                     axis=mybir.AxisListType.X)
            recip = work.tile([P, n_qt], F32, tag="recip")
            nc.vector.reciprocal(recip, dsum)

            # finalize: outT -> SBUF, transpose to [q, e], normalize, +bv
            osb = []
            for c in range(2):
                ot = outp.tile([P, QB], F32, tag="osb")
                nc.vector.tensor_copy(ot, po[c])
                osb.append(ot)
            for j in range(n_qt):
                res = outp.tile([P, EMB], F32, tag="res")
                for c in range(2):
                    otp = ps_tp.tile([P, P], F32, tag="tp")
                    nc.tensor.transpose(otp, osb[c][:, j * P:(j + 1) * P], idf)
                    nc.vector.tensor_scalar_mul(res[:, c * P:(c + 1) * P], otp,
                                                recip[:, j:j + 1])
                nc.vector.tensor_add(res, res, bv_bc)
                q0 = qb_i * QB + j * P
                nc.sync.dma_start(out[q0:q0 + P, :], res)


def _make_nc(s_len: int = S) -> bass.Bass:
    # Bacc (not raw Bass): its compile() splits multi-sem waits and moves
    # matmul waits onto ldweights — HW allows at most one wait per inst.
    nc = bacc.Bacc("TRN2", target_bir_lowering=False, debug=False)
    _build(nc, s_len)
    nc.compile()
    return nc


def _prep(inputs: dict) -> dict:
    arrs = {k: np.ascontiguousarray(np.asarray(v, dtype=np.float32))
            for k, v in inputs.items()}
    assert arrs["x"].shape == (B, S, EMB), arrs["x"].shape
    return arrs


def run(inputs: dict):
    """Run on 8 NeuronCores. Returns (out[B,S,E] f32, BassKernelResults)."""
    arrs = _prep(inputs)
    nc = _make_nc(S)
    shared = {k: arrs[k] for k in ("Wq", "bq", "Wk", "bk", "Wv", "bv")}
    in_maps = [dict(shared, x=arrs["x"][i]) for i in range(B)]
    res = bass_utils.run_bass_kernel_spmd(nc, in_maps, core_ids=list(range(B)))
    out = np.stack([r["out"] for r in res.results], axis=0).astype(np.float32)
    return out, res


def kernel(**inputs) -> np.ndarray:
    out, _ = run(inputs)
    return out


def bench(inputs: dict, iters: int = 5):
    """Compile once, then time repeated executions with device-resident
    inputs (mirrors bass2jax.run_bass_via_pjrt's multi-core path).

    Returns (out[B,S,E] f32, list of per-call wall times in seconds).
    """
    import time

    import jax
    from jax.sharding import Mesh, NamedSharding, PartitionSpec
    from jax.experimental.shard_map import shard_map

    from concourse import bass2jax
    from concourse import mybir as mb

    arrs = _prep(inputs)
    nc = _make_nc(S)
    bass2jax.install_neuronx_cc_hook()

    partition_name = (
        nc.partition_id_tensor.name if nc.partition_id_tensor else None
    )
    in_names, out_names, out_avals, zero_outs = [], [], [], []
    for alloc in nc.m.functions[0].allocations:
        if not isinstance(alloc, mb.MemoryLocationSet):
            continue
        name = alloc.memorylocations[0].name
        if alloc.kind == "ExternalInput":
            if name != partition_name:
                in_names.append(name)
        elif alloc.kind == "ExternalOutput":
            out_names.append(name)
            shape = tuple(alloc.tensor_shape)
            dtype = mb.dt.np(alloc.dtype)
            out_avals.append(jax.core.ShapedArray(shape, dtype))
            zero_outs.append(np.zeros(shape, dtype))
    n_params = len(in_names)
    n_outs = len(out_avals)
    all_names = in_names + out_names
    if partition_name is not None:
        all_names = all_names + [partition_name]

    def _body(*args):
        operands = list(args)
        if partition_name is not None:
            operands.append(bass2jax.partition_id_tensor())
        outs = bass2jax._bass_exec_p.bind(
            *operands,
            out_avals=tuple(out_avals),
            in_names=tuple(all_names),
            out_names=tuple(out_names),
            lowering_input_output_aliases=(),
            sim_require_finite=True,
            sim_require_nnan=True,
            nc=nc,
        )
        return tuple(outs)

    devices = jax.devices()[:B]
    mesh = Mesh(np.asarray(devices), ("core",))
    in_specs = (PartitionSpec("core"),) * (n_params + n_outs)
    out_specs = (PartitionSpec("core"),) * n_outs
    donate = tuple(range(n_params, n_params + n_outs))
    sharded = jax.jit(
        shard_map(_body, mesh=mesh, in_specs=in_specs, out_specs=out_specs,
                  check_rep=False),
        donate_argnums=donate,
        keep_unused=True,
    )

    per_core = [
        [arrs["x"][c] if n == "x" else arrs[n] for n in in_names[:n_params]]
        for c in range(B)
    ]
    concat_in = [
        np.concatenate([per_core[c][i] for c in range(B)], axis=0)
        for i in range(n_params)
    ]
    concat_zeros = [
        np.zeros((B * z.shape[0], *z.shape[1:]), z.dtype) for z in zero_outs
    ]

    shard = NamedSharding(mesh, PartitionSpec("core"))
    dev_in = [jax.device_put(a, shard) for a in concat_in]
    jax.block_until_ready(dev_in)

    times = []
    out_np = None
    for i in range(iters + 1):
        dev_zeros = [jax.device_put(z, shard) for z in concat_zeros]
        jax.block_until_ready(dev_zeros)
        t0 = time.perf_counter()
        outs = sharded(*dev_in, *dev_zeros)
        jax.block_until_ready(outs)
        dt = time.perf_counter() - t0
        if i == 0:
            idx = out_names.index("out")
            out_np = np.asarray(outs[idx]).reshape(B, S, EMB).astype(np.float32)
        else:
            times.append(dt)
    return out_np, times
